# revision 9
# baseline (speedup 1.0000x reference)
"""Bahdanau attention kernel for Trainium2 (8 NeuronCores, data-parallel over batch).

Computes, for inputs query[B,TQ,D], values[B,TV,D], enc_mask[B,TV] and params
W1[D,U], b1[U], W2[D,U], b2[U], Vw[U], Vb[]:

  q_proj = query @ W1 + b1                     # [B,TQ,U]
  v_proj = values @ W2 + b2                    # [B,TV,U]
  s      = tanh(q_proj[:,:,None,:] + v_proj[:,None,:,:])
  score  = s @ Vw + Vb - (1-mask)*1e9          # [B,TQ,TV]
  attn   = softmax(score, axis=-1)
  ctx    = attn @ values                       # [B,TQ,D]
  returns (ctx, attn)

Device strategy (per core, 2 batches, batches interleaved at q-group level):
  - u on partitions: q_projT [U,TQ], v_projT [U,TV] via PE matmuls (fp16 inputs,
    fp32 PSUM accumulate); query/values transposed via DMA-xbar (fp16)
  - broadcast add qp[u,q]+vp[u,v] = DVE tensor_scalar_add (per-partition scalar,
    fp16 -> 4x mode), batched into [128, 8192] arg tiles (8 q x 4 uc x 256 v)
  - tanh on ScalarE over the big tiles (amortizes per-inst overhead)
  - u-reduction on PE: tanh tile [128u,128v] as stationary (fp16 -> FWL),
    rhs = Vw chunk [128,1], accumulate over u-chunks into dense scoreT[v,q] PSUM
  - penalty (mask+Vb, host-prepped) as per-partition bias while draining scoreT
  - post-processing (softmax + context) per 32-query half-chunk to overlap
    with remaining tanh work; context accumulates into PSUM rows {0,32}
"""

import sys

if "/opt/trn_rl_repo" not in sys.path:
    sys.path.insert(0, "/opt/trn_rl_repo")

import numpy as np

B, TQ, TV, D, U = 16, 64, 256, 512, 512
NCORES = 8
BPC = B // NCORES  # batches per core
DC, UC, VC = D // 128, U // 128, TV // 128  # 4, 4, 2
QG = 8  # queries per arg super-tile
NQG = TQ // QG
QH = 32  # queries per post-processing half-chunk

TRACE = False
TRACE_ALL_CORES = False
LAST_EXEC_NS = None
LAST_RESULTS = None

_cache = {}


def _build_nc():
    import concourse.bass as bass  # noqa: F401
    import concourse.mybir as mybir
    import concourse.tile as tile
    from concourse import bacc
    from concourse.masks import make_identity

    f32 = mybir.dt.float32
    f16 = mybir.dt.float16

    nc = bacc.Bacc(None, target_bir_lowering=False)

    qf_h = nc.declare_dram_parameter("queryF", [BPC, TQ, D], f16, isOutput=False)
    v_h = nc.declare_dram_parameter("values", [BPC, TV, D], f32, isOutput=False)
    vf_h = nc.declare_dram_parameter("valuesF", [BPC, TV, D], f16, isOutput=False)
    pen_h = nc.declare_dram_parameter("pen", [BPC, TV], f32, isOutput=False)
    w1_h = nc.declare_dram_parameter("W1F", [D, U], f16, isOutput=False)
    w2_h = nc.declare_dram_parameter("W2F", [D, U], f16, isOutput=False)
    vw_h = nc.declare_dram_parameter("VwF", [U], f16, isOutput=False)
    b12_h = nc.declare_dram_parameter("b12", [U], f32, isOutput=False)
    ctx_h = nc.declare_dram_parameter("ctx_out", [BPC, TQ, D], f32, isOutput=True)
    att_h = nc.declare_dram_parameter("attn_out", [BPC, TQ, TV], f32, isOutput=True)

    with tile.TileContext(nc) as tc:
        with (
            tc.tile_pool(name="wp", bufs=1) as wp,
            tc.tile_pool(name="per_b", bufs=BPC) as pb,
            tc.tile_pool(name="big", bufs=4) as bigp,
            tc.tile_pool(name="small", bufs=2 * BPC) as smp,
            tc.tile_pool(name="half", bufs=4) as hp,
            tc.tile_pool(name="ps_vp", bufs=1, space="PSUM") as ps_vp,
            tc.tile_pool(name="ps_qp", bufs=1, space="PSUM") as ps_qp,
            tc.tile_pool(name="ps_tr", bufs=1, space="PSUM") as ps_tr,
            tc.tile_pool(name="ps_sT", bufs=2, space="PSUM") as ps_sT,
            tc.tile_pool(name="ps_sc", bufs=1, space="PSUM") as ps_sc,
            tc.tile_pool(name="ps_cx", bufs=1, space="PSUM") as ps_cx,
        ):
            # ---- constants / weights (loaded once) ----
            w2f = wp.tile([128, DC, U], f16, tag="w2")  # [p, dc, u] = W2[dc*128+p, u]
            nc.sync.dma_start(
                out=w2f[:], in_=w2_h[:].rearrange("(dc p) u -> p dc u", p=128)
            )
            w1f = wp.tile([128, DC, U], f16, tag="w1")
            nc.sync.dma_start(
                out=w1f[:], in_=w1_h[:].rearrange("(dc p) u -> p dc u", p=128)
            )
            vw_sb = wp.tile([128, UC], f16, tag="vw")  # [p, uc] = Vw[uc*128+p]
            nc.sync.dma_start(
                out=vw_sb[:], in_=vw_h[:].rearrange("(uc p) -> p uc", p=128)
            )
            b12_sb = wp.tile([128, UC], f32, tag="b12")
            nc.sync.dma_start(
                out=b12_sb[:], in_=b12_h[:].rearrange("(uc p) -> p uc", p=128)
            )
            ident = wp.tile([128, 128], f32, tag="ident")
            make_identity(nc, ident[:])

            prepped = [None] * BPC
            sT_tiles = [None] * BPC

            def emit_prep(b):
                # valuesT via DMA xbar: vTf[p, dc, v] = values[b, v, dc*128+p]
                vTf = pb.tile([128, DC, TV], f16, tag="vTf")
                for dc in range(DC):
                    nc.sync.dma_start_transpose(
                        out=vTf[:, dc, :], in_=vf_h[b][:, dc * 128 : (dc + 1) * 128]
                    )
                qTf = pb.tile([128, DC, TQ], f16, tag="qTf")
                for dc in range(DC):
                    nc.sync.dma_start_transpose(
                        out=qTf[:, dc, :], in_=qf_h[b][:, dc * 128 : (dc + 1) * 128]
                    )
                pen = pb.tile([128, VC], f32, tag="pen")  # [p, vc]
                nc.sync.dma_start(
                    out=pen[:], in_=pen_h[b].rearrange("(vc p) -> p vc", p=128)
                )
                vn = pb.tile([128, VC, D], f32, tag="vn")  # fp32, for context
                nc.sync.dma_start(
                    out=vn[:], in_=v_h[b].rearrange("(vc p) d -> p vc d", p=128)
                )

                # v_projT: vp[p, uc, v] = sum_d W2[d, uc*128+p] * values[b, v, d]
                vp_ps = ps_vp.tile([128, UC, TV], f32, tag="vp")
                for uc in range(UC):
                    for dc in range(DC):
                        nc.tensor.matmul(
                            vp_ps[:, uc, :],
                            lhsT=w2f[:, dc, uc * 128 : (uc + 1) * 128],
                            rhs=vTf[:, dc, :],
                            start=(dc == 0),
                            stop=(dc == DC - 1),
                        )
                vpF = pb.tile([128, UC, TV], f16, tag="vpF")
                for uc in range(UC):
                    nc.vector.tensor_copy(vpF[:, uc, :], vp_ps[:, uc, :])

                # q_projT (+ b1 + b2): qp[p, uc, q]
                qp_ps = ps_qp.tile([128, UC, TQ], f32, tag="qp")
                for uc in range(UC):
                    for dc in range(DC):
                        nc.tensor.matmul(
                            qp_ps[:, uc, :],
                            lhsT=w1f[:, dc, uc * 128 : (uc + 1) * 128],
                            rhs=qTf[:, dc, :],
                            start=(dc == 0),
                            stop=(dc == DC - 1),
                        )
                qpT = pb.tile([128, UC, TQ], f32, tag="qpT")
                for uc in range(UC):
                    nc.vector.tensor_scalar_add(
                        qpT[:, uc, :], qp_ps[:, uc, :], b12_sb[:, uc : uc + 1]
                    )

                prepped[b] = dict(vn=vn, pen=pen, vpF=vpF, qpT=qpT)
                sT_tiles[b] = ps_sT.tile([128, VC, TQ], f32, tag="sT", name="sT")

            def emit_qg(b, qg):
                pr = prepped[b]
                vpF, qpT = pr["vpF"], pr["qpT"]
                sT_ps = sT_tiles[b]
                arg = bigp.tile([128, QG * UC * 256], f16, tag="arg")
                for qq in range(QG):
                    q = qg * QG + qq
                    for uc in range(UC):
                        o = (qq * UC + uc) * 256
                        nc.vector.tensor_scalar_add(
                            arg[:, o : o + 256],
                            vpF[:, uc, :],
                            qpT[:, uc, q : q + 1],
                        )
                th = bigp.tile([128, QG * UC * 256], f16, tag="th")
                nc.scalar.activation(th[:], arg[:], mybir.ActivationFunctionType.Tanh)
                for qq in range(QG):
                    q = qg * QG + qq
                    for vc in range(VC):
                        for uc in range(UC):
                            o = (qq * UC + uc) * 256 + vc * 128
                            nc.tensor.matmul(
                                sT_ps[:, vc, q : q + 1],
                                lhsT=th[:, o : o + 128],
                                rhs=vw_sb[:, uc : uc + 1],
                                start=(uc == 0),
                                stop=(uc == UC - 1),
                            )

            cx_tiles = [None] * BPC

            def emit_post_half(b, h):
                """Softmax + attn + context rows for queries [h*QH, (h+1)*QH)."""
                pr = prepped[b]
                vn, pen = pr["vn"], pr["pen"]
                sT_ps = sT_tiles[b]
                qs = slice(h * QH, (h + 1) * QH)

                # drain scoreT chunk with penalty bias
                sTm = hp.tile([128, VC, QH], f32, tag="sTm")
                for vc in range(VC):
                    nc.scalar.activation(
                        sTm[:, vc, :],
                        sT_ps[:, vc, qs],
                        mybir.ActivationFunctionType.Identity,
                        bias=pen[:, vc : vc + 1],
                    )
                # transpose -> score[q, v] dense in PSUM
                sc_ps = ps_sc.tile([QH, TV], f32, tag="sc")
                for vc in range(VC):
                    nc.tensor.transpose(
                        sc_ps[:, vc * 128 : (vc + 1) * 128],
                        sTm[:, vc, :],
                        ident[:],
                    )
                # softmax over v (free dim)
                mx = hp.tile([QH, 1], f32, tag="mx")
                nc.vector.tensor_reduce(
                    mx[:], sc_ps[:], mybir.AxisListType.X, mybir.AluOpType.max
                )
                negm = hp.tile([QH, 1], f32, tag="negm")
                nc.vector.tensor_scalar_mul(negm[:], mx[:], -1.0)
                eS = hp.tile([QH, TV], f32, tag="eS")
                ssum = hp.tile([QH, 1], f32, tag="ssum")
                nc.scalar.activation(
                    eS[:],
                    sc_ps[:],
                    mybir.ActivationFunctionType.Exp,
                    bias=negm[:],
                    accum_out=ssum[:],
                )
                rsum = hp.tile([QH, 1], f32, tag="rsum")
                nc.vector.reciprocal(rsum[:], ssum[:])
                attn = hp.tile([QH, TV], f32, tag="attn")
                nc.vector.tensor_scalar_mul(attn[:], eS[:], rsum[:])
                nc.sync.dma_start(out=att_h[b][qs], in_=attn[:])

                # attnT chunk + context rows
                aT = hp.tile([128, VC, QH], f32, tag="aT")
                for vc in range(VC):
                    t = ps_tr.tile([128, 128], f32, tag="tr")
                    nc.tensor.transpose(
                        t[:, :QH], attn[:, vc * 128 : (vc + 1) * 128], ident[:QH, :QH]
                    )
                    nc.scalar.copy(aT[:, vc, :], t[:, :QH])
                if h == 0:
                    cx_tiles[b] = ps_cx.tile([TQ, D], f32, tag="cx", name="cx")
                cx_ps = cx_tiles[b]
                for vc in range(VC):
                    nc.tensor.matmul(
                        cx_ps[qs, :],
                        lhsT=aT[:, vc, :],
                        rhs=vn[:, vc, :],
                        start=(vc == 0),
                        stop=(vc == VC - 1),
                    )

            def emit_ctx_tail(b):
                cx_ps = cx_tiles[b]
                ctx_sb = smp.tile([TQ, D], f32, tag="ctx")
                nc.vector.tensor_copy(ctx_sb[:], cx_ps[:])
                nc.sync.dma_start(out=ctx_h[b], in_=ctx_sb[:])

            # ---- emission schedule: interleave the two batches ----
            def maybe_post(b, qg_done):
                if qg_done == NQG // 2 - 1:
                    emit_post_half(b, 0)
                elif qg_done == NQG - 1:
                    emit_post_half(b, 1)
                    emit_ctx_tail(b)

            emit_prep(0)
            emit_qg(0, 0)
            maybe_post(0, 0)
            emit_prep(1)
            for k in range(1, NQG):
                emit_qg(0, k)
                maybe_post(0, k)
                emit_qg(1, k - 1)
                maybe_post(1, k - 1)
            emit_qg(1, NQG - 1)
            maybe_post(1, NQG - 1)

    nc.finalize()
    return nc


def _get_nc():
    if "nc" not in _cache:
        _cache["nc"] = _build_nc()
    return _cache["nc"]


def kernel(query, values, enc_mask, W1, b1, W2, b2, Vw, Vb):
    global LAST_EXEC_NS, LAST_RESULTS
    from concourse.bass_utils import run_bass_kernel_spmd

    query = np.asarray(query, dtype=np.float32)
    values = np.ascontiguousarray(np.asarray(values, dtype=np.float32))
    enc_mask = np.asarray(enc_mask)
    queryF = np.ascontiguousarray(query.astype(np.float16))
    valuesF = np.ascontiguousarray(values.astype(np.float16))
    W1F = np.ascontiguousarray(np.asarray(W1, dtype=np.float32).astype(np.float16))
    W2F = np.ascontiguousarray(np.asarray(W2, dtype=np.float32).astype(np.float16))
    b12 = (np.asarray(b1, dtype=np.float32) + np.asarray(b2, dtype=np.float32))
    VwF = np.asarray(Vw, dtype=np.float32).astype(np.float16)
    vb = np.float32(np.asarray(Vb, dtype=np.float32))
    # additive mask penalty folded with Vb: score_final = score_raw + pen[v]
    pen = (vb - (1.0 - enc_mask.astype(np.float32)) * np.float32(1e9)).astype(
        np.float32
    )

    nc = _get_nc()
    in_maps = []
    for c in range(NCORES):
        s = slice(c * BPC, (c + 1) * BPC)
        in_maps.append(
            {
                "queryF": np.ascontiguousarray(queryF[s]),
                "values": np.ascontiguousarray(values[s]),
                "valuesF": np.ascontiguousarray(valuesF[s]),
                "pen": np.ascontiguousarray(pen[s]),
                "W1F": W1F,
                "W2F": W2F,
                "VwF": VwF,
                "b12": b12,
            }
        )

    kwargs = {}
    if TRACE:
        kwargs["trace"] = True
        if TRACE_ALL_CORES:
            kwargs["trace_cores"] = list(range(NCORES))
    res = run_bass_kernel_spmd(nc, in_maps, core_ids=list(range(NCORES)), **kwargs)
    LAST_RESULTS = res
    LAST_EXEC_NS = res.exec_time_ns

    ctx = np.concatenate([res.results[c]["ctx_out"] for c in range(NCORES)], axis=0)
    attn = np.concatenate([res.results[c]["attn_out"] for c in range(NCORES)], axis=0)
    return ctx.astype(np.float32), attn.astype(np.float32)


# revision 12
# speedup vs baseline: 1.0321x; 1.0321x over previous
"""Bahdanau attention kernel for Trainium2 (8 NeuronCores, data-parallel over batch).

Computes, for inputs query[B,TQ,D], values[B,TV,D], enc_mask[B,TV] and params
W1[D,U], b1[U], W2[D,U], b2[U], Vw[U], Vb[]:

  q_proj = query @ W1 + b1                     # [B,TQ,U]
  v_proj = values @ W2 + b2                    # [B,TV,U]
  s      = tanh(q_proj[:,:,None,:] + v_proj[:,None,:,:])
  score  = s @ Vw + Vb - (1-mask)*1e9          # [B,TQ,TV]
  attn   = softmax(score, axis=-1)
  ctx    = attn @ values                       # [B,TQ,D]
  returns (ctx, attn)

Device strategy (per core, 2 batches, batches interleaved at q-group level):
  - u on partitions: q_projT [U,TQ], v_projT [U,TV] via PE matmuls (fp16 inputs,
    fp32 PSUM accumulate); query/values transposed via DMA-xbar (fp16)
  - broadcast add qp[u,q]+vp[u,v] = DVE tensor_scalar_add (per-partition scalar,
    fp16 -> 4x mode), batched into [128, 8192] arg tiles (8 q x 4 uc x 256 v)
  - tanh on ScalarE over the big tiles (amortizes per-inst overhead)
  - u-reduction on PE: tanh tile [128u,128v] as stationary (fp16 -> FWL),
    rhs = Vw chunk [128,1], accumulate over u-chunks into dense scoreT[v,q] PSUM
  - penalty (mask+Vb, host-prepped) as per-partition bias while draining scoreT
  - post-processing (softmax + context) per 32-query half-chunk to overlap
    with remaining tanh work; context accumulates into PSUM rows {0,32}
"""

import sys

if "/opt/trn_rl_repo" not in sys.path:
    sys.path.insert(0, "/opt/trn_rl_repo")

import numpy as np

B, TQ, TV, D, U = 16, 64, 256, 512, 512
NCORES = 8
BPC = B // NCORES  # batches per core
DC, UC, VC = D // 128, U // 128, TV // 128  # 4, 4, 2
QG = 8  # queries per arg super-tile
NQG = TQ // QG
QH = 32  # queries per post-processing half-chunk

TRACE = False
TRACE_ALL_CORES = False
LAST_EXEC_NS = None
LAST_RESULTS = None

_cache = {}


def _build_nc():
    import concourse.bass as bass  # noqa: F401
    import concourse.mybir as mybir
    import concourse.tile as tile
    from concourse import bacc
    from concourse.masks import make_identity

    f32 = mybir.dt.float32
    f16 = mybir.dt.float16

    nc = bacc.Bacc(None, target_bir_lowering=False)

    qt_h = nc.declare_dram_parameter("queryT", [BPC, D, TQ], f16, isOutput=False)
    v_h = nc.declare_dram_parameter("values", [BPC, TV, D], f32, isOutput=False)
    vf_h = nc.declare_dram_parameter("valuesF", [BPC, TV, D], f16, isOutput=False)
    pen_h = nc.declare_dram_parameter("pen", [BPC, TV], f32, isOutput=False)
    w1_h = nc.declare_dram_parameter("W1F", [D, U], f16, isOutput=False)
    w2_h = nc.declare_dram_parameter("W2F", [D, U], f16, isOutput=False)
    vw_h = nc.declare_dram_parameter("VwF", [U], f16, isOutput=False)
    b12_h = nc.declare_dram_parameter("b12", [U], f32, isOutput=False)
    ctx_h = nc.declare_dram_parameter("ctx_out", [BPC, TQ, D], f32, isOutput=True)
    att_h = nc.declare_dram_parameter("attn_out", [BPC, TQ, TV], f32, isOutput=True)

    with tile.TileContext(nc) as tc:
        with (
            tc.tile_pool(name="wp", bufs=1) as wp,
            tc.tile_pool(name="per_b", bufs=BPC) as pb,
            tc.tile_pool(name="big", bufs=4) as bigp,
            tc.tile_pool(name="small", bufs=2 * BPC) as smp,
            tc.tile_pool(name="half", bufs=4) as hp,
            tc.tile_pool(name="ps_vp", bufs=1, space="PSUM") as ps_vp,
            tc.tile_pool(name="ps_qp", bufs=1, space="PSUM") as ps_qp,
            tc.tile_pool(name="ps_tr", bufs=1, space="PSUM") as ps_tr,
            tc.tile_pool(name="ps_sT", bufs=2, space="PSUM") as ps_sT,
            tc.tile_pool(name="ps_sc", bufs=1, space="PSUM") as ps_sc,
            tc.tile_pool(name="ps_cx", bufs=1, space="PSUM") as ps_cx,
        ):
            # ---- constants / weights (loaded once) ----
            w2f = wp.tile([128, DC, U], f16, tag="w2")  # [p, dc, u] = W2[dc*128+p, u]
            nc.gpsimd.dma_start(
                out=w2f[:], in_=w2_h[:].rearrange("(dc p) u -> p dc u", p=128)
            )
            w1f = wp.tile([128, DC, U], f16, tag="w1")
            nc.gpsimd.dma_start(
                out=w1f[:], in_=w1_h[:].rearrange("(dc p) u -> p dc u", p=128)
            )
            vw_sb = wp.tile([128, UC], f16, tag="vw")  # [p, uc] = Vw[uc*128+p]
            nc.gpsimd.dma_start(
                out=vw_sb[:], in_=vw_h[:].rearrange("(uc p) -> p uc", p=128)
            )
            b12_sb = wp.tile([128, UC], f32, tag="b12")
            nc.gpsimd.dma_start(
                out=b12_sb[:], in_=b12_h[:].rearrange("(uc p) -> p uc", p=128)
            )
            ident = wp.tile([128, 128], f32, tag="ident")
            make_identity(nc, ident[:])

            prepped = [None] * BPC
            sT_tiles = [None] * BPC

            def emit_prep(b):
                # valuesT via DMA xbar: vTf[p, dc, v] = values[b, v, dc*128+p]
                vTf = pb.tile([128, DC, TV], f16, tag="vTf")
                for dc in range(DC):
                    nc.sync.dma_start_transpose(
                        out=vTf[:, dc, :], in_=vf_h[b][:, dc * 128 : (dc + 1) * 128]
                    )
                qTf = pb.tile([128, DC, TQ], f16, tag="qTf")
                nc.gpsimd.dma_start(
                    out=qTf[:], in_=qt_h[b].rearrange("(dc p) q -> p dc q", p=128)
                )
                pen = pb.tile([128, VC], f32, tag="pen")  # [p, vc]
                nc.gpsimd.dma_start(
                    out=pen[:], in_=pen_h[b].rearrange("(vc p) -> p vc", p=128)
                )
                vn = pb.tile([128, VC, D], f32, tag="vn")  # fp32, for context
                nc.gpsimd.dma_start(
                    out=vn[:], in_=v_h[b].rearrange("(vc p) d -> p vc d", p=128)
                )

                # v_projT: vp[p, uc, v] = sum_d W2[d, uc*128+p] * values[b, v, d]
                vp_ps = ps_vp.tile([128, UC, TV], f32, tag="vp")
                for uc in range(UC):
                    for dc in range(DC):
                        nc.tensor.matmul(
                            vp_ps[:, uc, :],
                            lhsT=w2f[:, dc, uc * 128 : (uc + 1) * 128],
                            rhs=vTf[:, dc, :],
                            start=(dc == 0),
                            stop=(dc == DC - 1),
                        )
                vpF = pb.tile([128, UC, TV], f16, tag="vpF")
                for uc in range(UC):
                    nc.vector.tensor_copy(vpF[:, uc, :], vp_ps[:, uc, :])

                # q_projT (+ b1 + b2): qp[p, uc, q]
                qp_ps = ps_qp.tile([128, UC, TQ], f32, tag="qp")
                for uc in range(UC):
                    for dc in range(DC):
                        nc.tensor.matmul(
                            qp_ps[:, uc, :],
                            lhsT=w1f[:, dc, uc * 128 : (uc + 1) * 128],
                            rhs=qTf[:, dc, :],
                            start=(dc == 0),
                            stop=(dc == DC - 1),
                        )
                qpT = pb.tile([128, UC, TQ], f32, tag="qpT")
                for uc in range(UC):
                    nc.vector.tensor_scalar_add(
                        qpT[:, uc, :], qp_ps[:, uc, :], b12_sb[:, uc : uc + 1]
                    )

                prepped[b] = dict(vn=vn, pen=pen, vpF=vpF, qpT=qpT)
                sT_tiles[b] = ps_sT.tile([128, VC, TQ], f32, tag="sT", name="sT")

            def emit_qg(b, qg):
                pr = prepped[b]
                vpF, qpT = pr["vpF"], pr["qpT"]
                sT_ps = sT_tiles[b]
                arg = bigp.tile([128, QG * UC * 256], f16, tag="arg")
                for qq in range(QG):
                    q = qg * QG + qq
                    for uc in range(UC):
                        o = (qq * UC + uc) * 256
                        nc.vector.tensor_scalar_add(
                            arg[:, o : o + 256],
                            vpF[:, uc, :],
                            qpT[:, uc, q : q + 1],
                        )
                th = bigp.tile([128, QG * UC * 256], f16, tag="th")
                nc.scalar.activation(th[:], arg[:], mybir.ActivationFunctionType.Tanh)
                for qq in range(QG):
                    q = qg * QG + qq
                    for vc in range(VC):
                        for uc in range(UC):
                            o = (qq * UC + uc) * 256 + vc * 128
                            nc.tensor.matmul(
                                sT_ps[:, vc, q : q + 1],
                                lhsT=th[:, o : o + 128],
                                rhs=vw_sb[:, uc : uc + 1],
                                start=(uc == 0),
                                stop=(uc == UC - 1),
                            )

            cx_tiles = [None] * BPC

            def emit_post_half(b, h):
                """Softmax + attn + context rows for queries [h*QH, (h+1)*QH)."""
                pr = prepped[b]
                vn, pen = pr["vn"], pr["pen"]
                sT_ps = sT_tiles[b]
                qs = slice(h * QH, (h + 1) * QH)

                # drain scoreT chunk with penalty bias
                sTm = hp.tile([128, VC, QH], f32, tag="sTm")
                for vc in range(VC):
                    nc.scalar.activation(
                        sTm[:, vc, :],
                        sT_ps[:, vc, qs],
                        mybir.ActivationFunctionType.Identity,
                        bias=pen[:, vc : vc + 1],
                    )
                # transpose -> score[q, v] dense in PSUM
                sc_ps = ps_sc.tile([QH, TV], f32, tag="sc")
                for vc in range(VC):
                    nc.tensor.transpose(
                        sc_ps[:, vc * 128 : (vc + 1) * 128],
                        sTm[:, vc, :],
                        ident[:],
                    )
                # softmax over v (free dim)
                mx = hp.tile([QH, 1], f32, tag="mx")
                nc.vector.tensor_reduce(
                    mx[:], sc_ps[:], mybir.AxisListType.X, mybir.AluOpType.max
                )
                negm = hp.tile([QH, 1], f32, tag="negm")
                nc.vector.tensor_scalar_mul(negm[:], mx[:], -1.0)
                eS = hp.tile([QH, TV], f32, tag="eS")
                ssum = hp.tile([QH, 1], f32, tag="ssum")
                nc.scalar.activation(
                    eS[:],
                    sc_ps[:],
                    mybir.ActivationFunctionType.Exp,
                    bias=negm[:],
                    accum_out=ssum[:],
                )
                rsum = hp.tile([QH, 1], f32, tag="rsum")
                nc.vector.reciprocal(rsum[:], ssum[:])
                attn = hp.tile([QH, TV], f32, tag="attn")
                nc.vector.tensor_scalar_mul(attn[:], eS[:], rsum[:])
                nc.sync.dma_start(out=att_h[b][qs], in_=attn[:])

                # attnT chunk + context rows
                aT = hp.tile([128, VC, QH], f32, tag="aT")
                for vc in range(VC):
                    t = ps_tr.tile([128, 128], f32, tag="tr")
                    nc.tensor.transpose(
                        t[:, :QH], attn[:, vc * 128 : (vc + 1) * 128], ident[:QH, :QH]
                    )
                    nc.scalar.copy(aT[:, vc, :], t[:, :QH])
                if h == 0:
                    cx_tiles[b] = ps_cx.tile([TQ, D], f32, tag="cx", name="cx")
                cx_ps = cx_tiles[b]
                for vc in range(VC):
                    nc.tensor.matmul(
                        cx_ps[qs, :],
                        lhsT=aT[:, vc, :],
                        rhs=vn[:, vc, :],
                        start=(vc == 0),
                        stop=(vc == VC - 1),
                    )

            def emit_ctx_tail(b):
                cx_ps = cx_tiles[b]
                ctx_sb = smp.tile([TQ, D], f32, tag="ctx")
                nc.vector.tensor_copy(ctx_sb[:], cx_ps[:])
                nc.sync.dma_start(out=ctx_h[b], in_=ctx_sb[:])

            # ---- emission schedule: interleave the two batches ----
            def maybe_post(b, qg_done):
                if qg_done == NQG // 2 - 1:
                    emit_post_half(b, 0)
                elif qg_done == NQG - 1:
                    emit_post_half(b, 1)
                    emit_ctx_tail(b)

            emit_prep(0)
            emit_qg(0, 0)
            maybe_post(0, 0)
            emit_prep(1)
            for k in range(1, NQG):
                emit_qg(0, k)
                maybe_post(0, k)
                emit_qg(1, k - 1)
                maybe_post(1, k - 1)
            emit_qg(1, NQG - 1)
            maybe_post(1, NQG - 1)

    nc.finalize()
    return nc


def _get_nc():
    if "nc" not in _cache:
        _cache["nc"] = _build_nc()
    return _cache["nc"]


def kernel(query, values, enc_mask, W1, b1, W2, b2, Vw, Vb):
    global LAST_EXEC_NS, LAST_RESULTS
    from concourse.bass_utils import run_bass_kernel_spmd

    query = np.asarray(query, dtype=np.float32)
    values = np.ascontiguousarray(np.asarray(values, dtype=np.float32))
    enc_mask = np.asarray(enc_mask)
    queryT = np.ascontiguousarray(np.transpose(query.astype(np.float16), (0, 2, 1)))
    valuesF = np.ascontiguousarray(values.astype(np.float16))
    W1F = np.ascontiguousarray(np.asarray(W1, dtype=np.float32).astype(np.float16))
    W2F = np.ascontiguousarray(np.asarray(W2, dtype=np.float32).astype(np.float16))
    b12 = (np.asarray(b1, dtype=np.float32) + np.asarray(b2, dtype=np.float32))
    VwF = np.asarray(Vw, dtype=np.float32).astype(np.float16)
    vb = np.float32(np.asarray(Vb, dtype=np.float32))
    # additive mask penalty folded with Vb: score_final = score_raw + pen[v]
    pen = (vb - (1.0 - enc_mask.astype(np.float32)) * np.float32(1e9)).astype(
        np.float32
    )

    nc = _get_nc()
    in_maps = []
    for c in range(NCORES):
        s = slice(c * BPC, (c + 1) * BPC)
        in_maps.append(
            {
                "queryT": np.ascontiguousarray(queryT[s]),
                "values": np.ascontiguousarray(values[s]),
                "valuesF": np.ascontiguousarray(valuesF[s]),
                "pen": np.ascontiguousarray(pen[s]),
                "W1F": W1F,
                "W2F": W2F,
                "VwF": VwF,
                "b12": b12,
            }
        )

    kwargs = {}
    if TRACE:
        kwargs["trace"] = True
        if TRACE_ALL_CORES:
            kwargs["trace_cores"] = list(range(NCORES))
    res = run_bass_kernel_spmd(nc, in_maps, core_ids=list(range(NCORES)), **kwargs)
    LAST_RESULTS = res
    LAST_EXEC_NS = res.exec_time_ns

    ctx = np.concatenate([res.results[c]["ctx_out"] for c in range(NCORES)], axis=0)
    attn = np.concatenate([res.results[c]["attn_out"] for c in range(NCORES)], axis=0)
    return ctx.astype(np.float32), attn.astype(np.float32)
